# revision 16
# baseline (speedup 1.0000x reference)
"""CQAttention Trainium2 kernel (8-core data parallel).

Math (per example):
    S[i,j] = C@w_c [i] + Q@w_q [j] + (C*w_mul)@Q^T [i,j] + bias
    S1 = softmax_j(where(Qmask==0, -1e9, S))
    S2 = softmax_i(where(Cmask==0, -1e9, S))
    A  = S1 @ Q
    Bm = S1 @ S2^T @ C
    out = concat([C, A, C*A, C*Bm], axis=-1)

Key identities used:
  - softmax is shift-invariant: `bias` drops out entirely; per-row offsets
    drop out of the row softmax S1; per-column offsets drop out of S2.
  - With Qm'[d,j] = w_mul[d]*Q^T[d,j] + w_c[d], one weight matrix serves
    both score matmuls:
        E^T = exp(Qm'^T@CT + s1[j] + qneg[j])   [j part, i free]
              (s0[i] rides along free and cancels in the row softmax S1)
        Eu  = exp(CT_tile^T@Qm')                [i part, j free] (unmasked)
  - The C-side mask folds multiplicatively into the Traw rhs (host packs
    cm*C in bf16 together with a cm column):
        Traw|c = Eu^T @ [cm*C | cm]  ->  T' = Traw * (1/c)
  - Denominators come from augmented matmul columns:
        Araw|Bmraw|r = E^T.T @ [Q | T' | 1];  A = ..*(1/r), Bm = ..*(1/r)

Precision: score matmuls in fp16 (11-bit mantissa, fp32 PSUM accumulate),
post-exp matmuls and staging in bf16. The verbatim C columns of the output
are assembled on the host (pure memcpy of an input), as is the final f32
upcast/unpermute. Host passes pre-transposed/packed operand layouts.
"""

import os
import sys
from contextlib import ExitStack

import ml_dtypes
import numpy as np

for _p in ("/opt/trn_rl_repo", "/root/.axon_site/_ro/trn_rl_repo"):
    if os.path.isdir(_p) and _p not in sys.path:
        sys.path.append(_p)

import concourse.bass as bass
import concourse.tile as tile
from concourse import bacc, mybir
from concourse.bass import ds, ts
from concourse.bass_utils import run_bass_kernel_spmd

F32 = mybir.dt.float32
FP16 = mybir.dt.float16
BF16 = mybir.dt.bfloat16
AF = mybir.ActivationFunctionType
ALU = mybir.AluOpType

N_CORES = 8
B, LC, LQ, D = 64, 1024, 128, 128
B_LOC = B // N_CORES  # 8 examples per core
NT = LC // 128  # 8 Lc tiles of 128


def _build_graph():
    nc = bacc.Bacc("TRN2", target_bir_lowering=False, debug=False)

    CT = nc.dram_tensor("CT", [B_LOC, D, LC], FP16, kind="ExternalInput").ap()
    QT = nc.dram_tensor("QT", [B_LOC, D, LQ], FP16, kind="ExternalInput").ap()
    Qb = nc.dram_tensor("Qb", [B_LOC, LQ, D], BF16, kind="ExternalInput").ap()
    # host-packed, p-major: [p, t*130+x] = (cm*C)[t*128+p, x] | cm | 0
    Cmb = nc.dram_tensor("Cmb", [B_LOC, 128, NT * 130], BF16, kind="ExternalInput").ap()
    # host-packed, p-major unmasked C: [p, t*128+x] = C[t*128+p, x]
    Cub = nc.dram_tensor("Cub", [B_LOC, 128, LC], BF16, kind="ExternalInput").ap()
    Qneg = nc.dram_tensor("Qneg", [LQ, B_LOC], F32, kind="ExternalInput").ap()
    wmul = nc.dram_tensor("wmul", [D, 1], F32, kind="ExternalInput").ap()
    wc = nc.dram_tensor("wc", [D, 1], F32, kind="ExternalInput").ap()
    wq = nc.dram_tensor("wq", [D, 2], FP16, kind="ExternalInput").ap()
    # outputs, p-major: host unpermutes/upcasts and adds the C columns
    outA = nc.dram_tensor("outA", [B_LOC, 128, NT * 128], BF16, kind="ExternalOutput").ap()
    outCC = nc.dram_tensor("outCC", [B_LOC, 128, NT * 256], BF16, kind="ExternalOutput").ap()

    with tile.TileContext(nc) as tc:
        with ExitStack() as ctx:
            ep = ctx.enter_context

            const = ep(tc.tile_pool(name="const", bufs=1))
            p_ctall = ep(tc.tile_pool(name="ctall", bufs=B_LOC))
            p_cxb = ep(tc.tile_pool(name="cxb", bufs=B_LOC))
            p_cub = ep(tc.tile_pool(name="cub", bufs=B_LOC))
            p_small = ep(tc.tile_pool(name="small", bufs=40))
            p_qmt = ep(tc.tile_pool(name="qmt", bufs=B_LOC))
            p_qt = ep(tc.tile_pool(name="qt", bufs=B_LOC))
            p_eqt = ep(tc.tile_pool(name="eqt", bufs=B_LOC))
            p_ect = ep(tc.tile_pool(name="ect", bufs=8))
            p_abmr = ep(tc.tile_pool(name="abmr", bufs=B_LOC))
            p_stg = ep(tc.tile_pool(name="stg", bufs=3))
            p_scr = ep(tc.tile_pool(name="scr", bufs=3))

            # e1 and abm phases don't overlap much: share one 4-buf pool
            pp_mm = ep(tc.tile_pool(name="pp_mm", bufs=4, space="PSUM"))
            pp_e2 = ep(tc.tile_pool(name="pp_e2", bufs=2, space="PSUM"))
            pp_traw = ep(tc.tile_pool(name="pp_traw", bufs=2, space="PSUM"))

            wmul_sb = const.tile([D, 1], F32)
            nc.sync.dma_start(wmul_sb, wmul)
            wc_sb = const.tile([D, 1], F32)
            nc.sync.dma_start(wc_sb, wc)
            wq_sb = const.tile([D, 2], FP16)
            nc.sync.dma_start(wq_sb, wq)
            qneg_sb = const.tile([LQ, B_LOC], F32)
            nc.sync.dma_start(qneg_sb, Qneg)

            ct_alls, qt_sbs, cxbs, cubs, abm_rhss = [], [], [], [], []
            qm_ts, bias1s, eq_ts = [], [], []

            # ---- phase: loads (split across sync HWDGE and gpsimd SWDGE) ----
            for e in range(B_LOC):
                ct_all = p_ctall.tile([128, LC], FP16, tag="ctall")
                nc.sync.dma_start(ct_all, CT[e])
                qt_sb = p_qt.tile([128, LQ], FP16, tag="qt")
                nc.gpsimd.dma_start(qt_sb, QT[e])
                cxb = p_cxb.tile([128, NT * 130], BF16, tag="cxb")
                nc.sync.dma_start(cxb, Cmb[e])
                cub = p_cub.tile([128, LC], BF16, tag="cub")
                nc.gpsimd.dma_start(cub, Cub[e])
                abm_rhs = p_abmr.tile([128, 257], BF16, tag="abmr")
                nc.gpsimd.dma_start(abm_rhs[:, 0:128], Qb[e])
                nc.gpsimd.memset(abm_rhs[:, 256:257], 1.0)
                ct_alls.append(ct_all)
                qt_sbs.append(qt_sb)
                cxbs.append(cxb)
                cubs.append(cub)
                abm_rhss.append(abm_rhs)

            # ---- phase: Qm' = w_mul * Q^T + w_c, s1 column ----
            for e in range(B_LOC):
                qm_t = p_qmt.tile([128, 130], FP16, tag="qmt")
                nc.vector.tensor_scalar(
                    qm_t[:, 0:128],
                    qt_sbs[e],
                    wmul_sb,
                    wc_sb,
                    op0=ALU.mult,
                    op1=ALU.add,
                )
                nc.vector.tensor_copy(qm_t[:, 128:130], wq_sb)
                qm_ts.append(qm_t)

                s1_ps = pp_traw.tile([128, 2], F32, tag="ptraw")
                nc.tensor.matmul(s1_ps, lhsT=qt_sbs[e], rhs=wq_sb)
                bias1 = p_small.tile([128, 1], F32, tag="small")
                nc.vector.tensor_add(bias1, s1_ps[:, 0:1], qneg_sb[:, e : e + 1])
                bias1s.append(bias1)

            # ---- phase: E^T = exp(s2^T + s0 + s1 + qneg)  [j part, i free] ----
            for e in range(B_LOC):
                eq_t = p_eqt.tile([128, LC], BF16, tag="eqt")
                for h in range(2):
                    e1_ps = pp_mm.tile([128, 512], F32, tag="pmm")
                    nc.tensor.matmul(
                        e1_ps, lhsT=qm_ts[e][:, 0:128], rhs=ct_alls[e][:, ts(h, 512)]
                    )
                    nc.scalar.activation(
                        eq_t[:, ts(h, 512)],
                        e1_ps,
                        func=AF.Exp,
                        bias=bias1s[e],
                        scale=1.0,
                    )
                eq_ts.append(eq_t)

            # ---- phase: Eu = exp(s2 + s0) -> Traw|c -> T' (per example) ----
            for e in range(B_LOC):
                ec_pairs = []
                for pr in range(NT // 2):
                    e2_ps = pp_e2.tile([128, 260], F32, tag="pe2")
                    for k in range(2):
                        nc.tensor.matmul(
                            e2_ps[:, ds(130 * k, 130)],
                            lhsT=ct_alls[e][:, ts(2 * pr + k, 128)],
                            rhs=qm_ts[e][:, 0:130],
                        )
                    ecp = p_ect.tile([128, 2, 128], BF16, tag="ect")
                    nc.scalar.activation(
                        ecp,
                        e2_ps.rearrange("p (k x) -> p k x", k=2)[:, :, 0:128],
                        func=AF.Exp,
                    )
                    ec_pairs.append(ecp)

                traw_ps = pp_traw.tile([128, 129], F32, tag="ptraw")
                for t in range(NT):
                    nc.tensor.matmul(
                        traw_ps,
                        lhsT=ec_pairs[t // 2][:, t % 2, :],
                        rhs=cxbs[e][:, ds(130 * t, 129)],
                        start=(t == 0),
                        stop=(t == NT - 1),
                    )
                cinv = p_small.tile([128, 1], F32, tag="small")
                nc.vector.reciprocal(cinv, traw_ps[:, 128:129])
                nc.scalar.activation(
                    abm_rhss[e][:, 128:256],
                    traw_ps[:, 0:128],
                    func=AF.Copy,
                    scale=cinv,
                )

            # ---- phase: [Araw|Bmraw|r] matmuls + epilogue + stores ----
            for e in range(B_LOC):
                scrb = p_scr.tile([128, NT, 256], BF16, tag="scr")
                stg = p_stg.tile([128, NT, 256], BF16, tag="stg")
                for t in range(NT):
                    abm_ps = pp_mm.tile([128, 257], F32, tag="pmm")
                    nc.tensor.matmul(
                        abm_ps, lhsT=eq_ts[e][:, ts(t, 128)], rhs=abm_rhss[e]
                    )
                    rinv = p_small.tile([128, 1], F32, tag="small")
                    nc.vector.reciprocal(rinv, abm_ps[:, 256:257])
                    # [A|Bm] * (1/r), alternating DVE / ACT to balance load
                    if t % 2 == 0:
                        nc.vector.tensor_scalar_mul(
                            scrb[:, t, :], abm_ps[:, 0:256], rinv
                        )
                    else:
                        nc.scalar.activation(
                            scrb[:, t, :], abm_ps[:, 0:256], func=AF.Copy, scale=rinv
                        )
                    if t % 4 == 3:
                        # [C*A | C*Bm] for 4 tiles in one DVE op (bf16 4x),
                        # C doubled via step-0 middle dim
                        u = t - 3
                        cdup = bass.AP(
                            tensor=cubs[e].tensor,
                            offset=cubs[e][:, ts(u, 128)].offset,
                            ap=[cubs[e].ap[0], [128, 4], [0, 2], [1, 128]],
                        )
                        nc.vector.tensor_tensor(
                            stg[:, u : u + 4, :].rearrange(
                                "p t (k x) -> p t k x", k=2
                            ),
                            scrb[:, u : u + 4, :].rearrange(
                                "p t (k x) -> p t k x", k=2
                            ),
                            cdup,
                            op=ALU.mult,
                        )
                nc.sync.dma_start(
                    outA[e].rearrange("p (t x) -> p t x", x=128), scrb[:, :, 0:128]
                )
                nc.sync.dma_start(
                    outCC[e].rearrange("p (t x) -> p t x", x=256), stg
                )

    nc.compile()
    return nc


_GRAPH = None


def _graph():
    global _GRAPH
    if _GRAPH is None:
        _GRAPH = _build_graph()
    return _GRAPH


def make_in_maps(C, Q, Cmask, Qmask, w_c, w_q, w_mul):
    """Shard full inputs into per-core input maps (host-side layout prep)."""
    C = np.asarray(C, dtype=np.float32)
    Q = np.asarray(Q, dtype=np.float32)
    wmul_col = np.ascontiguousarray(np.asarray(w_mul, dtype=np.float32).reshape(D, 1))
    wc_col = np.ascontiguousarray(np.asarray(w_c, dtype=np.float32).reshape(D, 1))
    wq_col = np.asarray(w_q, dtype=np.float16).reshape(D, 1)
    wq2 = np.ascontiguousarray(np.concatenate([wq_col, wq_col], axis=1))
    in_maps = []
    for i in range(N_CORES):
        sl = slice(i * B_LOC, (i + 1) * B_LOC)
        qneg = (np.asarray(Qmask[sl], dtype=np.float32) - 1.0) * 1e9  # [8, 128]
        cm = np.asarray(Cmask[sl], dtype=np.float32)  # [8, 1024]
        Ci = C[sl]
        Qi = Q[sl]
        # p-major packed [e, p, t*130+x]
        cmb = np.zeros((B_LOC, LC, 130), dtype=ml_dtypes.bfloat16)
        cmb[:, :, 0:128] = (Ci * cm[:, :, None]).astype(ml_dtypes.bfloat16)
        cmb[:, :, 128] = cm.astype(ml_dtypes.bfloat16)
        cmb = np.ascontiguousarray(
            cmb.reshape(B_LOC, NT, 128, 130)
            .transpose(0, 2, 1, 3)
            .reshape(B_LOC, 128, NT * 130)
        )
        cub = np.ascontiguousarray(
            Ci.astype(ml_dtypes.bfloat16)
            .reshape(B_LOC, NT, 128, D)
            .transpose(0, 2, 1, 3)
            .reshape(B_LOC, 128, LC)
        )
        in_maps.append(
            {
                "CT": np.ascontiguousarray(Ci.transpose(0, 2, 1).astype(np.float16)),
                "QT": np.ascontiguousarray(Qi.transpose(0, 2, 1).astype(np.float16)),
                "Qb": np.ascontiguousarray(Qi.astype(ml_dtypes.bfloat16)),
                "Cmb": cmb,
                "Cub": cub,
                "Qneg": np.ascontiguousarray(qneg.T),  # [128, 8]
                "wmul": wmul_col,
                "wc": wc_col,
                "wq": wq2,
            }
        )
    return in_maps


def assemble(results, C):
    """Gather per-core device outputs + input C into the full f32 output."""
    out = np.empty((B, LC, 4 * D), dtype=np.float32)
    out[:, :, 0:D] = np.asarray(C, dtype=np.float32)
    for i in range(N_CORES):
        sl = slice(i * B_LOC, (i + 1) * B_LOC)
        a = results[i]["outA"]  # [B_LOC, 128, NT*128] bf16
        cc = results[i]["outCC"]  # [B_LOC, 128, NT*256] bf16
        out[sl, :, D : 2 * D] = (
            a.reshape(B_LOC, 128, NT, 128)
            .transpose(0, 2, 1, 3)
            .reshape(B_LOC, LC, 128)
            .astype(np.float32)
        )
        out[sl, :, 2 * D : 4 * D] = (
            cc.reshape(B_LOC, 128, NT, 2, 128)
            .transpose(0, 2, 1, 3, 4)
            .reshape(B_LOC, LC, 256)
            .astype(np.float32)
        )
    return out


def kernel(C, Q, Cmask, Qmask, w_c, w_q, w_mul, bias=None, **_ignored):
    # `bias` is mathematically a no-op: it shifts every score equally and
    # softmax is shift-invariant, so the output does not depend on it.
    nc = _graph()
    in_maps = make_in_maps(C, Q, Cmask, Qmask, w_c, w_q, w_mul)
    res = run_bass_kernel_spmd(nc, in_maps, core_ids=list(range(N_CORES)))
    return assemble(res.results, C)
